# revision 43
# baseline (speedup 1.0000x reference)
"""Trainium2 Bass kernel: FADEv4 retrieval-kNN head (nn_FADEv4_7026566496861).

Math (per image n):
    cls  = l2norm(mean_s(x_support_cls[n]))          # [1,D]
    q    = l2norm(x_query[n])                        # [Tq,D]
    s    = l2norm(x_support[n])                      # [Ts,D]
    sim  = q @ s.T                                   # [Tq,Ts]
    dmin = 1 - max_ts(sim); idx = argmax_ts(sim)
    pred = sigmoid(q@W1 + s[idx]@W2 + cls@W3 + b)
    out0 = (pred*dmin).reshape(N,1,37,37); out1 = pred.reshape(N,1,37,37)

Sharding: data-parallel over N=16 images -> 8 cores x 2 images, no collectives.

Kernel design (v2):
  * The sim matmul runs in fp8e4 with MatmulPerfMode.DoubleRow (2 k-tiles per
    pass at 0.5 cyc/row) -> 3 matmuls per [128,512] block instead of 6 bf16.
  * q is NOT normalized before the matmul (a positive per-row scale does not
    change the column argmax); 1/||q|| is folded into the dmin/p1 epilogue.
    s is scaled by 4/||s|| (column scale must precede the max).  W1/W2 are
    scaled by 16 to dodge fp8 subnormals; the head epilogue unscales.
  * normalize+cast fp8 is one fused DVE tensor_scalar pass (2x_2p); PE
    transposes the fp8 chunks; the transposed PSUM tiles are evacuated to
    SBUF by plain DMA (no scalar/DVE cost).
  * The scalar engine evacuates sim PSUM->SBUF as one bf16 row per m-block;
    the row max comes from a single tensor_mask_reduce (2x_1p on bf16) and
    the argmax from one full-row find_index8 (max_index).  No per-chunk
    max8/find/one-hot combine machinery.
  * p2 = s[idx]@W2 falls out of the matmul as an extra query column; it is
    staged to DRAM (scaled 1/64, bf16) and gathered per row by indirect DMA.
"""

import os
from contextlib import ExitStack

import numpy as np

import concourse.bass as bass
import concourse.mybir as mybir
import concourse.tile as tile
from concourse import bacc
from concourse.bass import ds, IndirectOffsetOnAxis
from concourse.bass_utils import run_bass_kernel_spmd
from concourse.masks import make_identity

F32 = mybir.dt.float32
BF16 = mybir.dt.bfloat16
F8 = mybir.dt.float8e4
U32 = mybir.dt.uint32
I16 = mybir.dt.int16
AX = mybir.AxisListType
OP = mybir.AluOpType
ACTF = mybir.ActivationFunctionType
DR = mybir.MatmulPerfMode.DoubleRow

N_FULL, TQ, TS, S, D = 16, 1369, 5476, 4, 768
SIDE = 37
KC = D // 128              # 6 contraction k-tiles
W2COL = 1376               # W2 lives at qT column 1376 (cols 1369..1375 zero)
TQE = W2COL + 1            # 1377 logical qT columns
TQAL = TQE + 3             # tile width padded to 4 so fp8 weight planes stay
                           # 4-byte aligned (ldweights ISA requirement)
MB = (TQE + 127) // 128    # 11 m-blocks (last: 97 cols, 89 real queries)
NJ = 11                    # 11 support j-blocks of 512
SIMW = NJ * 512            # 5632 (supports 0..5475, W1 col at 5476, pad after)
JGROUPS = [(0, 3), (3, 3), (6, 3), (9, 2)]
NEG = -1.0e30

N_CORES = 8
PER_CORE = N_FULL // N_CORES

# FADE_MM: "f8dr" (fp8e4 + DoubleRow) or "bf16" (6 plain k-tile matmuls)
MM_MODE = os.environ.get("FADE_MM", "f8dr")
# FADE_TEVAC: "dve" (u16-bitcast copy) or "scalar" evacuation of transposes
TEVAC = os.environ.get("FADE_TEVAC", "dve")
# FADE_VAL: "mask" (tensor_mask_reduce row max) or "max8"
VALMODE = os.environ.get("FADE_VAL", "max8")
# FADE_STAGE: 1=mm+evac 2=+rowmax 3=+find 4=+gather 5=full
STAGE = int(os.environ.get("FADE_STAGE", "5"))
# FADE_POOL: "max8" = full-row max8+find_index8 on DVE;
#            "gd" = windowed pool split gpsimd/DVE + window gather;
#            "gp"/"dve" = windowed pool all on one engine
POOLMODE = os.environ.get("FADE_POOL", "gd")
WIN = 64                   # pool window size
NWIN = SIMW // WIN         # 88 windows


def _build_T(nc, pools, consts, src_dram, tok0, rows, dstT, dst_off, scale_mode,
             mm_dtype, qscales=None, m=None):
    """DMA one 128-row chunk, compute norms, cast (scaled) to mm_dtype,
    PE-transpose into dstT[:, k, dst_off:dst_off+rows]."""
    (raw_pool, f8_pool, scratch, psum_t, dummy_sq) = pools
    (identm,) = consts

    raw = raw_pool.tile([128, D], F32, tag="nt_raw")
    nc.sync.dma_start(out=raw[:rows, :], in_=src_dram[ds(tok0, rows), :])
    ssn = scratch.tile([128, 4], F32, tag="nt_ss")
    nc.scalar.activation(
        dummy_sq[:rows, :], raw[:rows, :], ACTF.Square, accum_out=ssn[:rows, 0:1]
    )
    cast = f8_pool.tile([128, D], mm_dtype, tag="nt_cast")
    if scale_mode == "s":
        # r = 4/||s||: sqrt(ssq/16) = ||s||/4, then reciprocal
        nc.scalar.activation(ssn[:rows, 1:2], ssn[:rows, 0:1], ACTF.Sqrt,
                             scale=1.0 / 16.0)
        nc.vector.reciprocal(ssn[:rows, 2:3], ssn[:rows, 1:2])
        nc.vector.tensor_scalar_mul(cast[:rows, :], raw[:rows, :], ssn[:rows, 2:3])
    else:
        # q: plain cast; store -1/(4||q||) and 1/(16||q||) for the epilogue
        nc.scalar.activation(ssn[:rows, 1:2], ssn[:rows, 0:1], ACTF.Sqrt,
                             scale=16.0)
        nc.vector.reciprocal(ssn[:rows, 2:3], ssn[:rows, 1:2])
        nrq4, p1s = qscales
        nc.vector.tensor_scalar_mul(nrq4[:rows, m:m + 1], ssn[:rows, 2:3], -1.0)
        nc.vector.tensor_scalar_mul(p1s[:rows, m:m + 1], ssn[:rows, 2:3], 0.25)
        nc.vector.tensor_copy(cast[:rows, :], raw[:rows, :])

    if mm_dtype == F8:
        # Transpose fp8 PAIRS as bf16 bit patterns: partition p of block b
        # then holds dims (256b+2p, 256b+2p+1) interleaved along tokens.
        # The DoubleRow ifmap (sT) may be pair-interleaved, so sT keeps this
        # layout; the weights (qT) must be plane-separated, so the q path
        # de-interleaves with two strided byte copies.
        cast16 = cast.bitcast(BF16)          # [128, 384]
        pst = psum_t.tile([128, KC // 2, 128, 2], mm_dtype, tag="nt_ps")
        for b in range(KC // 2):
            nc.tensor.transpose(
                pst[:, b, :rows, :].bitcast(BF16)
                .rearrange("p r c -> p (r c)"),
                cast16[:rows, ds(b * 128, 128)], identm[:rows, :rows]
            )
        if scale_mode == "s":
            # packed 2-byte evacuation (2x_1p) on the DVE (rows here are
            # always even: full 128-chunks plus a final 100-row chunk)
            nc.vector.tensor_copy(
                dstT[:, :, ds(dst_off, rows), :].bitcast(I16)
                .rearrange("p a b c -> p a (b c)"),
                pst[:, :, :rows, :].bitcast(I16)
                .rearrange("p a b c -> p a (b c)"),
            )
        else:
            # de-interleave into per-m-block planes: the dual-fp8 ldweights
            # ISA wants the two weight planes near-adjacent, so qT is
            # [128, 3, MB, 2, 128] with plane stride 128.
            c = dst_off // 128
            for kk in range(2):
                nc.vector.tensor_copy(
                    dstT[:, :, c, kk, :rows],
                    pst[:, :, :rows, kk],
                )
    else:
        pst = psum_t.tile([128, KC, 128], mm_dtype, tag="nt_ps")
        for k in range(KC):
            nc.tensor.transpose(
                pst[:, k, :rows], cast[:rows, ds(k * 128, 128)],
                identm[:rows, :rows]
            )
        nc.scalar.copy(dstT[:, :, ds(dst_off, rows)], pst[:, :, :rows])


def _emit_image(nc, pools, consts, aps, n, mm_dtype):
    (img_pool, raw_pool, f8_pool, scratch, simb_pool, psum_t, psum_mm,
     dummy_sq, cls_pool) = pools
    (identm, w1m, w2m, w3, bh, e5476, iota88) = consts
    (x_query, x_support, x_cls, p2d_list, c3d_list, simd_list, out0, out1) = aps

    # ---- cls head scalar: c3 = (sum_cls . W3)/||sum_cls|| + b ----
    clsbig = cls_pool.tile([1, S * D], F32, tag="clsbig")
    nc.sync.dma_start(out=clsbig[:, :], in_=x_cls[n])
    clsum = scratch.tile([1, D], F32, tag="clsum")
    nc.vector.tensor_add(clsum[:, :], clsbig[:, 0:D], clsbig[:, D:2 * D])
    nc.vector.tensor_add(clsum[:, :], clsum[:, :], clsbig[:, 2 * D:3 * D])
    nc.vector.tensor_add(clsum[:, :], clsum[:, :], clsbig[:, 3 * D:4 * D])
    sc3 = scratch.tile([1, D], F32, tag="sc3")
    ss3 = scratch.tile([1, 8], F32, tag="ss3")
    nc.vector.tensor_mul(sc3[:, :], clsum[:, :], clsum[:, :])
    nc.vector.tensor_reduce(out=ss3[:, 0:1], in_=sc3[:, :], axis=AX.X, op=OP.add)
    nc.vector.tensor_mul(sc3[:, :], clsum[:, :], w3[:, :])
    nc.vector.tensor_reduce(out=ss3[:, 1:2], in_=sc3[:, :], axis=AX.X, op=OP.add)
    nc.scalar.sqrt(ss3[:, 2:3], ss3[:, 0:1])
    nc.vector.reciprocal(ss3[:, 3:4], ss3[:, 2:3])
    nc.vector.tensor_mul(ss3[:, 4:5], ss3[:, 1:2], ss3[:, 3:4])
    nc.vector.tensor_add(ss3[:, 5:6], ss3[:, 4:5], bh[:, 0:1])
    nc.sync.dma_start(out=c3d_list[n][:, :], in_=ss3[0:1, 5:6])
    c3b = img_pool.tile([128, 1], F32, tag="c3b")
    nc.sync.dma_start(out=c3b[:, :], in_=c3d_list[n][:, :].to_broadcast((128, 1)))

    # ---- build qT (fp8: [128, 3, MB, 2, 128] per-m-block pair planes;
    #      bf16: [128, KC, TQAL]) and sT (fp8: interleaved
    #      [128, KC/2, SIMW, 2]; bf16: [128, KC, SIMW]) ----
    if mm_dtype == F8:
        qT = img_pool.tile([128, KC // 2, MB, 2, 128], mm_dtype, tag="qT",
                           name="qT")
    else:
        qT = img_pool.tile([128, KC, TQAL], mm_dtype, tag="qT", name="qT")
    nrq4 = img_pool.tile([128, MB], F32, tag="nrq4")
    p1s = img_pool.tile([128, MB], F32, tag="p1s")
    bpools = (raw_pool, f8_pool, scratch, psum_t, dummy_sq)
    for c in range(MB):
        tok0 = c * 128
        rows = min(128, TQ - tok0)
        if rows > 0:
            _build_T(nc, bpools, (identm,), x_query[n], tok0, rows, qT, tok0,
                     "q", mm_dtype, qscales=(nrq4, p1s), m=c)
    if mm_dtype == F8:
        # zero the pad columns 1369..1375 (m-block 10, local 89..95)
        nc.vector.memset(qT[:, :, MB - 1, :, 89:96], 0)
        for k in range(KC):
            nc.vector.tensor_copy(
                qT[:, k // 2, MB - 1, k % 2, 96:97], w2m[:, k:k + 1]
            )
    else:
        nc.vector.memset(qT[:, :, TQ:W2COL], 0)
        for k in range(KC):
            nc.vector.tensor_copy(qT[:, k, W2COL:W2COL + 1], w2m[:, k:k + 1])

    if mm_dtype == F8:
        sT = img_pool.tile([128, KC // 2, SIMW, 2], mm_dtype, tag="sT", name="sT")
    else:
        sT = img_pool.tile([128, KC, SIMW], mm_dtype, tag="sT", name="sT")
    off = 0
    while off < TS:
        rows = min(128, TS - off)
        _build_T(nc, bpools, (identm,), x_support[n], off, rows, sT, off,
                 "s", mm_dtype)
        off += rows
    if mm_dtype == F8:
        for k in range(KC):
            nc.vector.tensor_copy(
                sT[:, k // 2, TS:TS + 1, k % 2], w1m[:, k:k + 1]
            )
        nc.vector.memset(sT[:, :, TS + 1:SIMW, :], 0)
    else:
        for k in range(KC):
            nc.vector.tensor_copy(sT[:, k, TS:TS + 1], w1m[:, k:k + 1])
        nc.vector.memset(sT[:, :, TS + 1:SIMW], 0)
    return (qT, sT, nrq4, p1s, c3b)


def _emit_sim(nc, pools, consts, aps, n, built, mm_dtype):
    (img_pool, raw_pool, f8_pool, scratch, simb_pool, psum_t, psum_mm,
     dummy_sq, cls_pool) = pools
    (identm, w1m, w2m, w3, bh, e5476, iota88) = consts
    (x_query, x_support, x_cls, p2d_list, c3d_list, simd_list, out0, out1) = aps
    (qT, sT, nrq4, p1s, c3b) = built

    # Per-m results accumulate into [128, MB] tiles; ALL dependent work
    # (window finds, p2 gathers, head) is batched after the m-loop so the
    # in-order scalar/DVE queues never stall mid-loop on gather latency.
    gm_all = img_pool.tile([128, MB], F32, tag="gm_all")
    wp_all = img_pool.tile([128, MB], F32, tag="wp_all")    # pure w*
    go_all = img_pool.tile([128, MB], F32, tag="go_all")    # global row offset
    p1_all = img_pool.tile([128, MB], F32, tag="p1_all")
    win_all = img_pool.tile([128, MB, WIN], BF16, tag="win_all")

    # m = MB-1 runs FIRST: it computes the p2 row (W2 column), which must be
    # staged to DRAM before the batched p2 gather runs.
    for m in [MB - 1] + list(range(MB - 1)):
        mcols = 128 if m < MB - 1 else TQE - 128 * (MB - 1)   # 97 on last
        mreal = 128 if m < MB - 1 else TQ - 128 * (MB - 1)    # 89 on last
        simb = simb_pool.tile([128, SIMW], BF16, tag="simb")
        for (j0, cnt) in JGROUPS:
            bp = psum_mm.tile([128, 3, 512], F32, tag="bp")
            for jj in range(cnt):
                j = j0 + jj
                if mm_dtype == F8:
                    for kp in range(3):
                        nc.tensor.matmul(
                            bp[:mcols, jj, :],
                            lhsT=qT[:, kp, m, :, :mcols],
                            rhs=sT[:, kp, ds(j * 512, 512), :]
                            .rearrange("p n two -> p two n"),
                            start=(kp == 0), stop=(kp == 2), perf_mode=DR,
                        )
                else:
                    for k in range(KC):
                        nc.tensor.matmul(
                            bp[:mcols, jj, :],
                            lhsT=qT[:, k, ds(m * 128, mcols)],
                            rhs=sT[:, k, ds(j * 512, 512)],
                            start=(k == 0), stop=(k == KC - 1),
                        )
            width = cnt * 512
            mev = mcols if m == MB - 1 else mreal
            nc.scalar.copy(
                simb[:mev, ds(j0 * 512, width)],
                bp[:mev, :cnt, :].rearrange("p a b -> p (a b)"),
            )
        if m == MB - 1:
            # p2 row (x64 scale; the gather epilogue folds in 1/64)
            nc.gpsimd.dma_start(
                out=p2d_list[n][ds(0, TS), 0], in_=simb[96:97, 0:TS]
            )

        # p1 column, then mask the tail and stage the row block to DRAM
        nc.scalar.copy(p1_all[:mreal, m:m + 1], simb[:mreal, TS:TS + 1])
        nc.vector.memset(simb[:, TS:SIMW], NEG)
        nc.gpsimd.dma_start(
            out=simd_list[n][ds(m * 128 * NWIN, 128 * NWIN), :],
            in_=simb[:, :],
        )

        # windowed row max on the DVE; winning window via max_index
        wmax = scratch.tile([128, NWIN], F32, tag="wmax")
        nc.vector.tensor_reduce(
            out=wmax[:, :], in_=simb[:, :].rearrange("p (w c) -> p w c", c=WIN),
            axis=AX.X, op=OP.max,
        )
        gm8f = scratch.tile([128, 8], F32, tag="gm8f")
        nc.vector.max(gm8f[:, :], wmax[:, :])
        nc.vector.tensor_copy(gm_all[:, m:m + 1], gm8f[:, 0:1])
        w8 = scratch.tile([128, 8], U32, tag="w8")
        nc.vector.max_index(w8[:, :], gm8f[:, :], wmax[:, :])
        nc.vector.tensor_copy(wp_all[:, m:m + 1], w8[:, 0:1])
        nc.vector.tensor_scalar(
            out=go_all[:, m:m + 1], in0=wp_all[:, m:m + 1],
            scalar1=float(m * 128 * NWIN), scalar2=iota88[:, 0:1],
            op0=OP.add, op1=OP.add,
        )

    # ---- batched window gather + finds ----
    gou = scratch.tile([128, MB], U32, tag="gou")
    nc.vector.tensor_copy(gou[:, :], go_all[:, :])
    for m in range(MB):
        nc.gpsimd.indirect_dma_start(
            out=win_all[:, m, :], out_offset=None, in_=simd_list[n][:, :],
            in_offset=IndirectOffsetOnAxis(ap=gou[:, m:m + 1], axis=0),
        )
    pos_all = scratch.tile([128, MB], F32, tag="pos_all")
    for m in range(MB):
        w8b = scratch.tile([128, 8], BF16, tag="w8b")
        nc.vector.max(w8b[:, :], win_all[:, m, :])
        pos8 = scratch.tile([128, 8], U32, tag="pos8")
        nc.vector.max_index(pos8[:, :], w8b[:, :], win_all[:, m, :])
        nc.vector.tensor_copy(pos_all[:, m:m + 1], pos8[:, 0:1])
    idxf = scratch.tile([128, MB], F32, tag="idxf")
    nc.vector.tensor_scalar_mul(idxf[:, :], wp_all[:, :], float(WIN))
    nc.vector.tensor_add(idxf[:, :], idxf[:, :], pos_all[:, :])
    gidxu = scratch.tile([128, MB], U32, tag="gidxu")
    nc.vector.tensor_copy(gidxu[:, :], idxf[:, :])

    # ---- batched p2 gather + head ----
    p2g = scratch.tile([128, MB], BF16, tag="p2g")
    for m in range(MB):
        nc.gpsimd.indirect_dma_start(
            out=p2g[:, m:m + 1], out_offset=None, in_=p2d_list[n][:, :],
            in_offset=IndirectOffsetOnAxis(ap=gidxu[:, m:m + 1], axis=0),
        )
    lg = scratch.tile([128, MB], F32, tag="lg")
    nc.vector.tensor_mul(lg[:, :], p1_all[:, :], p1s[:, :])
    p2f = scratch.tile([128, MB], F32, tag="p2f")
    nc.vector.tensor_scalar_mul(p2f[:, :], p2g[:, :], 1.0 / 64.0)
    nc.vector.tensor_add(lg[:, :], lg[:, :], p2f[:, :])
    pred = scratch.tile([128, MB], F32, tag="pred")
    nc.scalar.activation(pred[:, :], lg[:, :], ACTF.Sigmoid, bias=c3b[:, 0:1])
    dmv = scratch.tile([128, MB], F32, tag="dmv")
    nc.vector.tensor_mul(dmv[:, :], gm_all[:, :], nrq4[:, :])
    nc.vector.tensor_scalar_add(dmv[:, :], dmv[:, :], 1.0)
    o0 = scratch.tile([128, MB], F32, tag="o0")
    nc.vector.tensor_mul(o0[:, :], pred[:, :], dmv[:, :])
    for m in range(MB):
        mreal = 128 if m < MB - 1 else TQ - 128 * (MB - 1)
        nc.gpsimd.dma_start(out=out1[n, ds(m * 128, mreal)], in_=pred[:mreal, m:m + 1])
        nc.gpsimd.dma_start(out=out0[n, ds(m * 128, mreal)], in_=o0[:mreal, m:m + 1])


def build_program(per_core=PER_CORE, mm_mode=MM_MODE):
    mm_dtype = F8 if mm_mode == "f8dr" else BF16
    nc = bacc.Bacc("TRN2", target_bir_lowering=False, debug=False)
    x_query = nc.dram_tensor("x_query", [per_core, TQ, D], F32, kind="ExternalInput").ap()
    x_support = nc.dram_tensor("x_support", [per_core, TS, D], F32, kind="ExternalInput").ap()
    x_cls = nc.dram_tensor("x_support_cls", [per_core, S * D], F32, kind="ExternalInput").ap()
    w_head = nc.dram_tensor("W_head", [3 * D, 1], F32, kind="ExternalInput").ap()
    b_head = nc.dram_tensor("b_head", [1, 1], F32, kind="ExternalInput").ap()
    out0 = nc.dram_tensor("out0", [per_core, TQ], F32, kind="ExternalOutput").ap()
    out1 = nc.dram_tensor("out1", [per_core, TQ], F32, kind="ExternalOutput").ap()
    p2d_list = [nc.dram_tensor(f"p2d_{i}", [SIMW, 1], BF16).ap() for i in range(per_core)]
    simd_list = [
        nc.dram_tensor(f"simd_{i}", [MB * 128 * NWIN, WIN], BF16).ap()
        for i in range(per_core)
    ]
    c3d_list = [nc.dram_tensor(f"c3d_{i}", [1, 1], F32).ap() for i in range(per_core)]

    with tile.TileContext(nc) as tc, ExitStack() as ctx:
        img_pool = ctx.enter_context(
            tc.tile_pool(name="img", bufs=2 if mm_dtype == F8 else 1)
        )
        raw_pool = ctx.enter_context(tc.tile_pool(name="raw", bufs=4))
        f8_pool = ctx.enter_context(tc.tile_pool(name="f8", bufs=4))
        scratch = ctx.enter_context(tc.tile_pool(name="scratch", bufs=4))
        simb_pool = ctx.enter_context(tc.tile_pool(name="simb", bufs=int(os.environ.get("FADE_SIMBBUFS", "4"))))
        cls_pool = ctx.enter_context(tc.tile_pool(name="cls", bufs=1))
        const_pool = ctx.enter_context(tc.tile_pool(name="const", bufs=1))
        psum_t = ctx.enter_context(tc.tile_pool(name="psum_t", bufs=2, space="PSUM"))
        psum_mm = ctx.enter_context(tc.tile_pool(name="psum_mm", bufs=2, space="PSUM"))

        # constants
        # fp8 mode: the transpose works on u16 fp8-pairs, so the identity is
        # u16 and the W1/W2 columns use the pair-interleaved dim order
        # (column 2b+kk holds W[256b + 2p + kk]).
        identm = const_pool.tile(
            [128, 128], BF16 if mm_dtype == F8 else mm_dtype
        )
        make_identity(nc, identm[:, :])
        e5476 = const_pool.tile([128, 1], F32)
        nc.vector.memset(e5476[:, :], float(TS))
        w1s = const_pool.tile([128, KC], F32)
        w2s = const_pool.tile([128, KC], F32)
        w1m = const_pool.tile([128, KC], mm_dtype)
        w2m = const_pool.tile([128, KC], mm_dtype)
        w3 = const_pool.tile([1, D], F32)
        bh = const_pool.tile([1, 1], F32)
        if mm_dtype == F8:
            for b in range(KC // 2):
                nc.sync.dma_start(
                    out=w1s[:, 2 * b:2 * b + 2],
                    in_=w_head[ds(256 * b, 256), :]
                    .rearrange("(p two) one -> p (two one)", two=2),
                )
                nc.sync.dma_start(
                    out=w2s[:, 2 * b:2 * b + 2],
                    in_=w_head[ds(D + 256 * b, 256), :]
                    .rearrange("(p two) one -> p (two one)", two=2),
                )
        else:
            for k in range(KC):
                nc.sync.dma_start(out=w1s[:, k:k + 1], in_=w_head[ds(128 * k, 128), :])
                nc.sync.dma_start(out=w2s[:, k:k + 1], in_=w_head[ds(D + 128 * k, 128), :])
        nc.scalar.mul(w1m[:, :], w1s[:, :], 16.0)
        nc.scalar.mul(w2m[:, :], w2s[:, :], 16.0)
        nc.sync.dma_start(out=w3[0:1, :], in_=w_head[ds(2 * D, D), :])
        nc.sync.dma_start(out=bh[:, :], in_=b_head[:, :])

        dummy_sq = const_pool.tile([128, D], F32)

        pools = (img_pool, raw_pool, f8_pool, scratch, simb_pool, psum_t,
                 psum_mm, dummy_sq, cls_pool)
        iota88u = const_pool.tile([128, 1], U32)
        iota88 = const_pool.tile([128, 1], F32)
        nc.gpsimd.iota(iota88u[:, :], pattern=[[0, 1]], base=0,
                       channel_multiplier=NWIN)
        nc.vector.tensor_copy(iota88[:, :], iota88u[:, :])
        consts = (identm, w1m, w2m, w3, bh, e5476, iota88)
        aps = (x_query, x_support, x_cls, p2d_list, c3d_list, simd_list, out0, out1)

        built = [None] * per_core
        for i in range(per_core):
            built[i] = _emit_image(nc, pools, consts, aps, i, mm_dtype)
        for i in range(per_core):
            _emit_sim(nc, pools, consts, aps, i, built[i], mm_dtype)

    nc.compile()
    return nc


_CACHED = {}


def _get_program(per_core=PER_CORE, mm_mode=MM_MODE):
    key = (per_core, mm_mode)
    if key not in _CACHED:
        _CACHED[key] = build_program(per_core, mm_mode)
    return _CACHED[key]


def run(inputs, trace=False, per_core=PER_CORE, mm_mode=MM_MODE):
    nc = _get_program(per_core, mm_mode)
    n_cores = N_FULL // per_core
    xq = np.ascontiguousarray(inputs["x_query"], dtype=np.float32)
    xs = np.ascontiguousarray(inputs["x_support"], dtype=np.float32)
    xc = np.ascontiguousarray(inputs["x_support_cls"], dtype=np.float32).reshape(
        N_FULL, S * D
    )
    wh = np.ascontiguousarray(inputs["W_head"], dtype=np.float32).reshape(3 * D, 1)
    bhv = np.ascontiguousarray(inputs["b_head"], dtype=np.float32).reshape(1, 1)
    in_maps = []
    for c in range(n_cores):
        sl = slice(c * per_core, (c + 1) * per_core)
        in_maps.append({
            "x_query": xq[sl], "x_support": xs[sl], "x_support_cls": xc[sl],
            "W_head": wh, "b_head": bhv,
        })
    res = run_bass_kernel_spmd(nc, in_maps, list(range(n_cores)), trace=trace)
    o0 = np.concatenate([res.results[c]["out0"] for c in range(n_cores)], axis=0)
    o1 = np.concatenate([res.results[c]["out1"] for c in range(n_cores)], axis=0)
    o0 = o0.reshape(N_FULL, 1, SIDE, SIDE).astype(np.float32)
    o1 = o1.reshape(N_FULL, 1, SIDE, SIDE).astype(np.float32)
    return (o0, o1), res


def kernel(**inputs):
    (o0, o1), _ = run(inputs, trace=False)
    return o0, o1


# revision 44
# speedup vs baseline: 1.0032x; 1.0032x over previous
"""Trainium2 Bass kernel: FADEv4 retrieval-kNN head (nn_FADEv4_7026566496861).

Math (per image n):
    cls  = l2norm(mean_s(x_support_cls[n]))          # [1,D]
    q    = l2norm(x_query[n])                        # [Tq,D]
    s    = l2norm(x_support[n])                      # [Ts,D]
    sim  = q @ s.T                                   # [Tq,Ts]
    dmin = 1 - max_ts(sim); idx = argmax_ts(sim)
    pred = sigmoid(q@W1 + s[idx]@W2 + cls@W3 + b)
    out0 = (pred*dmin).reshape(N,1,37,37); out1 = pred.reshape(N,1,37,37)

Sharding: data-parallel over N=16 images -> 8 cores x 2 images, no collectives.

Kernel design (v2):
  * The sim matmul runs in fp8e4 with MatmulPerfMode.DoubleRow (2 k-tiles per
    pass at 0.5 cyc/row) -> 3 matmuls per [128,512] block instead of 6 bf16.
  * q is NOT normalized before the matmul (a positive per-row scale does not
    change the column argmax); 1/||q|| is folded into the dmin/p1 epilogue.
    s is scaled by 4/||s|| (column scale must precede the max).  W1/W2 are
    scaled by 16 to dodge fp8 subnormals; the head epilogue unscales.
  * normalize+cast fp8 is one fused DVE tensor_scalar pass (2x_2p); PE
    transposes the fp8 chunks; the transposed PSUM tiles are evacuated to
    SBUF by plain DMA (no scalar/DVE cost).
  * The scalar engine evacuates sim PSUM->SBUF as one bf16 row per m-block;
    the row max comes from a single tensor_mask_reduce (2x_1p on bf16) and
    the argmax from one full-row find_index8 (max_index).  No per-chunk
    max8/find/one-hot combine machinery.
  * p2 = s[idx]@W2 falls out of the matmul as an extra query column; it is
    staged to DRAM (scaled 1/64, bf16) and gathered per row by indirect DMA.
"""

import os
from contextlib import ExitStack

import numpy as np

import concourse.bass as bass
import concourse.mybir as mybir
import concourse.tile as tile
from concourse import bacc
from concourse.bass import ds, IndirectOffsetOnAxis
from concourse.bass_utils import run_bass_kernel_spmd
from concourse.masks import make_identity

F32 = mybir.dt.float32
BF16 = mybir.dt.bfloat16
F8 = mybir.dt.float8e4
U32 = mybir.dt.uint32
I16 = mybir.dt.int16
AX = mybir.AxisListType
OP = mybir.AluOpType
ACTF = mybir.ActivationFunctionType
DR = mybir.MatmulPerfMode.DoubleRow

N_FULL, TQ, TS, S, D = 16, 1369, 5476, 4, 768
SIDE = 37
KC = D // 128              # 6 contraction k-tiles
W2COL = 1376               # W2 lives at qT column 1376 (cols 1369..1375 zero)
TQE = W2COL + 1            # 1377 logical qT columns
TQAL = TQE + 3             # tile width padded to 4 so fp8 weight planes stay
                           # 4-byte aligned (ldweights ISA requirement)
MB = (TQE + 127) // 128    # 11 m-blocks (last: 97 cols, 89 real queries)
NJ = 11                    # 11 support j-blocks of 512
SIMW = NJ * 512            # 5632 (supports 0..5475, W1 col at 5476, pad after)
JGROUPS = [(0, 3), (3, 3), (6, 3), (9, 2)]
NEG = -1.0e30

N_CORES = 8
PER_CORE = N_FULL // N_CORES

if os.environ.get("FADE_LDWOPT", "0") == "1":
    import concourse.bass_utils as _bu_patch

    if not getattr(_bu_patch, "_fade_ldwopt", False):
        _orig_run_command = _bu_patch.run_command

        def _run_command_ldwopt(cmd, *a, **k):
            cmd = [
                c.replace("--enable-ldw-opt=false", "--enable-ldw-opt=true")
                if isinstance(c, str) else c
                for c in cmd
            ]
            return _orig_run_command(cmd, *a, **k)

        _bu_patch.run_command = _run_command_ldwopt
        _bu_patch._fade_ldwopt = True

# FADE_MM: "f8dr" (fp8e4 + DoubleRow) or "bf16" (6 plain k-tile matmuls)
MM_MODE = os.environ.get("FADE_MM", "f8dr")
# FADE_TEVAC: "dve" (u16-bitcast copy) or "scalar" evacuation of transposes
TEVAC = os.environ.get("FADE_TEVAC", "dve")
# FADE_VAL: "mask" (tensor_mask_reduce row max) or "max8"
VALMODE = os.environ.get("FADE_VAL", "max8")
# FADE_STAGE: 1=mm+evac 2=+rowmax 3=+find 4=+gather 5=full
STAGE = int(os.environ.get("FADE_STAGE", "5"))
# FADE_POOL: "max8" = full-row max8+find_index8 on DVE;
#            "gd" = windowed pool split gpsimd/DVE + window gather;
#            "gp"/"dve" = windowed pool all on one engine
POOLMODE = os.environ.get("FADE_POOL", "gd")
WIN = 64                   # pool window size
NWIN = SIMW // WIN         # 88 windows


def _build_T(nc, pools, consts, src_dram, tok0, rows, dstT, dst_off, scale_mode,
             mm_dtype, qscales=None, m=None):
    """DMA one 128-row chunk, compute norms, cast (scaled) to mm_dtype,
    PE-transpose into dstT[:, k, dst_off:dst_off+rows]."""
    (raw_pool, f8_pool, scratch, psum_t, dummy_sq) = pools
    (identm,) = consts

    raw = raw_pool.tile([128, D], F32, tag="nt_raw")
    nc.sync.dma_start(out=raw[:rows, :], in_=src_dram[ds(tok0, rows), :])
    ssn = scratch.tile([128, 4], F32, tag="nt_ss")
    nc.scalar.activation(
        dummy_sq[:rows, :], raw[:rows, :], ACTF.Square, accum_out=ssn[:rows, 0:1]
    )
    cast = f8_pool.tile([128, D], mm_dtype, tag="nt_cast")
    if scale_mode == "s":
        # r = 4/||s||: sqrt(ssq/16) = ||s||/4, then reciprocal
        nc.scalar.activation(ssn[:rows, 1:2], ssn[:rows, 0:1], ACTF.Sqrt,
                             scale=1.0 / 16.0)
        nc.vector.reciprocal(ssn[:rows, 2:3], ssn[:rows, 1:2])
        nc.vector.tensor_scalar_mul(cast[:rows, :], raw[:rows, :], ssn[:rows, 2:3])
    else:
        # q: plain cast; store -1/(4||q||) and 1/(16||q||) for the epilogue
        nc.scalar.activation(ssn[:rows, 1:2], ssn[:rows, 0:1], ACTF.Sqrt,
                             scale=16.0)
        nc.vector.reciprocal(ssn[:rows, 2:3], ssn[:rows, 1:2])
        nrq4, p1s = qscales
        nc.vector.tensor_scalar_mul(nrq4[:rows, m:m + 1], ssn[:rows, 2:3], -1.0)
        nc.vector.tensor_scalar_mul(p1s[:rows, m:m + 1], ssn[:rows, 2:3], 0.25)
        nc.vector.tensor_copy(cast[:rows, :], raw[:rows, :])

    if mm_dtype == F8:
        # Transpose fp8 PAIRS as bf16 bit patterns: partition p of block b
        # then holds dims (256b+2p, 256b+2p+1) interleaved along tokens.
        # The DoubleRow ifmap (sT) may be pair-interleaved, so sT keeps this
        # layout; the weights (qT) must be plane-separated, so the q path
        # de-interleaves with two strided byte copies.
        cast16 = cast.bitcast(BF16)          # [128, 384]
        pst = psum_t.tile([128, KC // 2, 128, 2], mm_dtype, tag="nt_ps")
        for b in range(KC // 2):
            nc.tensor.transpose(
                pst[:, b, :rows, :].bitcast(BF16)
                .rearrange("p r c -> p (r c)"),
                cast16[:rows, ds(b * 128, 128)], identm[:rows, :rows]
            )
        if scale_mode == "s":
            # packed 2-byte evacuation (2x_1p) on the DVE (rows here are
            # always even: full 128-chunks plus a final 100-row chunk)
            nc.vector.tensor_copy(
                dstT[:, :, ds(dst_off, rows), :].bitcast(I16)
                .rearrange("p a b c -> p a (b c)"),
                pst[:, :, :rows, :].bitcast(I16)
                .rearrange("p a b c -> p a (b c)"),
            )
        else:
            # de-interleave into per-m-block planes: the dual-fp8 ldweights
            # ISA wants the two weight planes near-adjacent, so qT is
            # [128, 3, MB, 2, 128] with plane stride 128.
            c = dst_off // 128
            for kk in range(2):
                nc.vector.tensor_copy(
                    dstT[:, :, c, kk, :rows],
                    pst[:, :, :rows, kk],
                )
    else:
        pst = psum_t.tile([128, KC, 128], mm_dtype, tag="nt_ps")
        for k in range(KC):
            nc.tensor.transpose(
                pst[:, k, :rows], cast[:rows, ds(k * 128, 128)],
                identm[:rows, :rows]
            )
        nc.scalar.copy(dstT[:, :, ds(dst_off, rows)], pst[:, :, :rows])


def _emit_image(nc, pools, consts, aps, n, mm_dtype):
    (img_pool, raw_pool, f8_pool, scratch, simb_pool, psum_t, psum_mm,
     dummy_sq, cls_pool) = pools
    (identm, w1m, w2m, w3, bh, e5476, iota88) = consts
    (x_query, x_support, x_cls, p2d_list, c3d_list, simd_list, out0, out1) = aps

    # ---- cls head scalar: c3 = (sum_cls . W3)/||sum_cls|| + b ----
    clsbig = cls_pool.tile([1, S * D], F32, tag="clsbig")
    nc.sync.dma_start(out=clsbig[:, :], in_=x_cls[n])
    clsum = scratch.tile([1, D], F32, tag="clsum")
    nc.vector.tensor_add(clsum[:, :], clsbig[:, 0:D], clsbig[:, D:2 * D])
    nc.vector.tensor_add(clsum[:, :], clsum[:, :], clsbig[:, 2 * D:3 * D])
    nc.vector.tensor_add(clsum[:, :], clsum[:, :], clsbig[:, 3 * D:4 * D])
    sc3 = scratch.tile([1, D], F32, tag="sc3")
    ss3 = scratch.tile([1, 8], F32, tag="ss3")
    nc.vector.tensor_mul(sc3[:, :], clsum[:, :], clsum[:, :])
    nc.vector.tensor_reduce(out=ss3[:, 0:1], in_=sc3[:, :], axis=AX.X, op=OP.add)
    nc.vector.tensor_mul(sc3[:, :], clsum[:, :], w3[:, :])
    nc.vector.tensor_reduce(out=ss3[:, 1:2], in_=sc3[:, :], axis=AX.X, op=OP.add)
    nc.scalar.sqrt(ss3[:, 2:3], ss3[:, 0:1])
    nc.vector.reciprocal(ss3[:, 3:4], ss3[:, 2:3])
    nc.vector.tensor_mul(ss3[:, 4:5], ss3[:, 1:2], ss3[:, 3:4])
    nc.vector.tensor_add(ss3[:, 5:6], ss3[:, 4:5], bh[:, 0:1])
    nc.sync.dma_start(out=c3d_list[n][:, :], in_=ss3[0:1, 5:6])
    c3b = img_pool.tile([128, 1], F32, tag="c3b")
    nc.sync.dma_start(out=c3b[:, :], in_=c3d_list[n][:, :].to_broadcast((128, 1)))

    # ---- build qT (fp8: [128, 3, MB, 2, 128] per-m-block pair planes;
    #      bf16: [128, KC, TQAL]) and sT (fp8: interleaved
    #      [128, KC/2, SIMW, 2]; bf16: [128, KC, SIMW]) ----
    if mm_dtype == F8:
        qT = img_pool.tile([128, KC // 2, MB, 2, 128], mm_dtype, tag="qT",
                           name="qT")
    else:
        qT = img_pool.tile([128, KC, TQAL], mm_dtype, tag="qT", name="qT")
    nrq4 = img_pool.tile([128, MB], F32, tag="nrq4")
    p1s = img_pool.tile([128, MB], F32, tag="p1s")
    bpools = (raw_pool, f8_pool, scratch, psum_t, dummy_sq)
    for c in range(MB):
        tok0 = c * 128
        rows = min(128, TQ - tok0)
        if rows > 0:
            _build_T(nc, bpools, (identm,), x_query[n], tok0, rows, qT, tok0,
                     "q", mm_dtype, qscales=(nrq4, p1s), m=c)
    if mm_dtype == F8:
        # zero the pad columns 1369..1375 (m-block 10, local 89..95)
        nc.vector.memset(qT[:, :, MB - 1, :, 89:96], 0)
        for k in range(KC):
            nc.vector.tensor_copy(
                qT[:, k // 2, MB - 1, k % 2, 96:97], w2m[:, k:k + 1]
            )
    else:
        nc.vector.memset(qT[:, :, TQ:W2COL], 0)
        for k in range(KC):
            nc.vector.tensor_copy(qT[:, k, W2COL:W2COL + 1], w2m[:, k:k + 1])

    if mm_dtype == F8:
        sT = img_pool.tile([128, KC // 2, SIMW, 2], mm_dtype, tag="sT", name="sT")
    else:
        sT = img_pool.tile([128, KC, SIMW], mm_dtype, tag="sT", name="sT")
    off = 0
    while off < TS:
        rows = min(128, TS - off)
        _build_T(nc, bpools, (identm,), x_support[n], off, rows, sT, off,
                 "s", mm_dtype)
        off += rows
    if mm_dtype == F8:
        for k in range(KC):
            nc.vector.tensor_copy(
                sT[:, k // 2, TS:TS + 1, k % 2], w1m[:, k:k + 1]
            )
        nc.vector.memset(sT[:, :, TS + 1:SIMW, :], 0)
    else:
        for k in range(KC):
            nc.vector.tensor_copy(sT[:, k, TS:TS + 1], w1m[:, k:k + 1])
        nc.vector.memset(sT[:, :, TS + 1:SIMW], 0)
    return (qT, sT, nrq4, p1s, c3b)


def _emit_sim(nc, pools, consts, aps, n, built, mm_dtype):
    (img_pool, raw_pool, f8_pool, scratch, simb_pool, psum_t, psum_mm,
     dummy_sq, cls_pool) = pools
    (identm, w1m, w2m, w3, bh, e5476, iota88) = consts
    (x_query, x_support, x_cls, p2d_list, c3d_list, simd_list, out0, out1) = aps
    (qT, sT, nrq4, p1s, c3b) = built

    # Per-m results accumulate into [128, MB] tiles; ALL dependent work
    # (window finds, p2 gathers, head) is batched after the m-loop so the
    # in-order scalar/DVE queues never stall mid-loop on gather latency.
    gm_all = img_pool.tile([128, MB], F32, tag="gm_all")
    wp_all = img_pool.tile([128, MB], F32, tag="wp_all")    # pure w*
    go_all = img_pool.tile([128, MB], F32, tag="go_all")    # global row offset
    p1_all = img_pool.tile([128, MB], F32, tag="p1_all")
    win_all = img_pool.tile([128, MB, WIN], BF16, tag="win_all")

    # m = MB-1 runs FIRST: it computes the p2 row (W2 column), which must be
    # staged to DRAM before the batched p2 gather runs.
    for m in [MB - 1] + list(range(MB - 1)):
        mcols = 128 if m < MB - 1 else TQE - 128 * (MB - 1)   # 97 on last
        mreal = 128 if m < MB - 1 else TQ - 128 * (MB - 1)    # 89 on last
        simb = simb_pool.tile([128, SIMW], BF16, tag="simb")
        for (j0, cnt) in JGROUPS:
            bp = psum_mm.tile([128, 3, 512], F32, tag="bp")
            # kp-outer: consecutive matmuls share the same weights so the
            # walrus ldw-opt (when enabled) can elide redundant LDWEIGHTS
            if mm_dtype == F8:
                for kp in range(3):
                    for jj in range(cnt):
                        nc.tensor.matmul(
                            bp[:mcols, jj, :],
                            lhsT=qT[:, kp, m, :, :mcols],
                            rhs=sT[:, kp, ds((j0 + jj) * 512, 512), :]
                            .rearrange("p n two -> p two n"),
                            start=(kp == 0), stop=(kp == 2), perf_mode=DR,
                            skip_group_check=True,
                        )
            else:
                for k in range(KC):
                    for jj in range(cnt):
                        nc.tensor.matmul(
                            bp[:mcols, jj, :],
                            lhsT=qT[:, k, ds(m * 128, mcols)],
                            rhs=sT[:, k, ds((j0 + jj) * 512, 512)],
                            start=(k == 0), stop=(k == KC - 1),
                            skip_group_check=True,
                        )
            width = cnt * 512
            mev = mcols if m == MB - 1 else mreal
            nc.scalar.copy(
                simb[:mev, ds(j0 * 512, width)],
                bp[:mev, :cnt, :].rearrange("p a b -> p (a b)"),
            )
        if m == MB - 1:
            # p2 row (x64 scale; the gather epilogue folds in 1/64)
            nc.gpsimd.dma_start(
                out=p2d_list[n][ds(0, TS), 0], in_=simb[96:97, 0:TS]
            )

        # p1 column, then mask the tail and stage the row block to DRAM
        nc.scalar.copy(p1_all[:mreal, m:m + 1], simb[:mreal, TS:TS + 1])
        nc.vector.memset(simb[:, TS:SIMW], NEG)
        nc.gpsimd.dma_start(
            out=simd_list[n][ds(m * 128 * NWIN, 128 * NWIN), :],
            in_=simb[:, :],
        )

        # windowed row max on the DVE; winning window via max_index
        wmax = scratch.tile([128, NWIN], F32, tag="wmax")
        nc.vector.tensor_reduce(
            out=wmax[:, :], in_=simb[:, :].rearrange("p (w c) -> p w c", c=WIN),
            axis=AX.X, op=OP.max,
        )
        gm8f = scratch.tile([128, 8], F32, tag="gm8f")
        nc.vector.max(gm8f[:, :], wmax[:, :])
        nc.vector.tensor_copy(gm_all[:, m:m + 1], gm8f[:, 0:1])
        w8 = scratch.tile([128, 8], U32, tag="w8")
        nc.vector.max_index(w8[:, :], gm8f[:, :], wmax[:, :])
        nc.vector.tensor_copy(wp_all[:, m:m + 1], w8[:, 0:1])
        nc.vector.tensor_scalar(
            out=go_all[:, m:m + 1], in0=wp_all[:, m:m + 1],
            scalar1=float(m * 128 * NWIN), scalar2=iota88[:, 0:1],
            op0=OP.add, op1=OP.add,
        )

    # ---- batched window gather + finds ----
    gou = scratch.tile([128, MB], U32, tag="gou")
    nc.vector.tensor_copy(gou[:, :], go_all[:, :])
    for m in range(MB):
        nc.gpsimd.indirect_dma_start(
            out=win_all[:, m, :], out_offset=None, in_=simd_list[n][:, :],
            in_offset=IndirectOffsetOnAxis(ap=gou[:, m:m + 1], axis=0),
        )
    pos_all = scratch.tile([128, MB], F32, tag="pos_all")
    for m in range(MB):
        w8b = scratch.tile([128, 8], BF16, tag="w8b")
        nc.vector.max(w8b[:, :], win_all[:, m, :])
        pos8 = scratch.tile([128, 8], U32, tag="pos8")
        nc.vector.max_index(pos8[:, :], w8b[:, :], win_all[:, m, :])
        nc.vector.tensor_copy(pos_all[:, m:m + 1], pos8[:, 0:1])
    idxf = scratch.tile([128, MB], F32, tag="idxf")
    nc.vector.tensor_scalar_mul(idxf[:, :], wp_all[:, :], float(WIN))
    nc.vector.tensor_add(idxf[:, :], idxf[:, :], pos_all[:, :])
    gidxu = scratch.tile([128, MB], U32, tag="gidxu")
    nc.vector.tensor_copy(gidxu[:, :], idxf[:, :])

    # ---- batched p2 gather + head ----
    p2g = scratch.tile([128, MB], BF16, tag="p2g")
    for m in range(MB):
        nc.gpsimd.indirect_dma_start(
            out=p2g[:, m:m + 1], out_offset=None, in_=p2d_list[n][:, :],
            in_offset=IndirectOffsetOnAxis(ap=gidxu[:, m:m + 1], axis=0),
        )
    lg = scratch.tile([128, MB], F32, tag="lg")
    nc.vector.tensor_mul(lg[:, :], p1_all[:, :], p1s[:, :])
    p2f = scratch.tile([128, MB], F32, tag="p2f")
    nc.vector.tensor_scalar_mul(p2f[:, :], p2g[:, :], 1.0 / 64.0)
    nc.vector.tensor_add(lg[:, :], lg[:, :], p2f[:, :])
    pred = scratch.tile([128, MB], F32, tag="pred")
    nc.scalar.activation(pred[:, :], lg[:, :], ACTF.Sigmoid, bias=c3b[:, 0:1])
    dmv = scratch.tile([128, MB], F32, tag="dmv")
    nc.vector.tensor_mul(dmv[:, :], gm_all[:, :], nrq4[:, :])
    nc.vector.tensor_scalar_add(dmv[:, :], dmv[:, :], 1.0)
    o0 = scratch.tile([128, MB], F32, tag="o0")
    nc.vector.tensor_mul(o0[:, :], pred[:, :], dmv[:, :])
    for m in range(MB):
        mreal = 128 if m < MB - 1 else TQ - 128 * (MB - 1)
        nc.gpsimd.dma_start(out=out1[n, ds(m * 128, mreal)], in_=pred[:mreal, m:m + 1])
        nc.gpsimd.dma_start(out=out0[n, ds(m * 128, mreal)], in_=o0[:mreal, m:m + 1])


def build_program(per_core=PER_CORE, mm_mode=MM_MODE):
    mm_dtype = F8 if mm_mode == "f8dr" else BF16
    nc = bacc.Bacc("TRN2", target_bir_lowering=False, debug=False)
    x_query = nc.dram_tensor("x_query", [per_core, TQ, D], F32, kind="ExternalInput").ap()
    x_support = nc.dram_tensor("x_support", [per_core, TS, D], F32, kind="ExternalInput").ap()
    x_cls = nc.dram_tensor("x_support_cls", [per_core, S * D], F32, kind="ExternalInput").ap()
    w_head = nc.dram_tensor("W_head", [3 * D, 1], F32, kind="ExternalInput").ap()
    b_head = nc.dram_tensor("b_head", [1, 1], F32, kind="ExternalInput").ap()
    out0 = nc.dram_tensor("out0", [per_core, TQ], F32, kind="ExternalOutput").ap()
    out1 = nc.dram_tensor("out1", [per_core, TQ], F32, kind="ExternalOutput").ap()
    p2d_list = [nc.dram_tensor(f"p2d_{i}", [SIMW, 1], BF16).ap() for i in range(per_core)]
    simd_list = [
        nc.dram_tensor(f"simd_{i}", [MB * 128 * NWIN, WIN], BF16).ap()
        for i in range(per_core)
    ]
    c3d_list = [nc.dram_tensor(f"c3d_{i}", [1, 1], F32).ap() for i in range(per_core)]

    with tile.TileContext(nc) as tc, ExitStack() as ctx:
        img_pool = ctx.enter_context(
            tc.tile_pool(name="img", bufs=2 if mm_dtype == F8 else 1)
        )
        raw_pool = ctx.enter_context(tc.tile_pool(name="raw", bufs=4))
        f8_pool = ctx.enter_context(tc.tile_pool(name="f8", bufs=4))
        scratch = ctx.enter_context(tc.tile_pool(name="scratch", bufs=4))
        simb_pool = ctx.enter_context(tc.tile_pool(name="simb", bufs=int(os.environ.get("FADE_SIMBBUFS", "4"))))
        cls_pool = ctx.enter_context(tc.tile_pool(name="cls", bufs=1))
        const_pool = ctx.enter_context(tc.tile_pool(name="const", bufs=1))
        psum_t = ctx.enter_context(tc.tile_pool(name="psum_t", bufs=2, space="PSUM"))
        psum_mm = ctx.enter_context(tc.tile_pool(name="psum_mm", bufs=2, space="PSUM"))

        # constants
        # fp8 mode: the transpose works on u16 fp8-pairs, so the identity is
        # u16 and the W1/W2 columns use the pair-interleaved dim order
        # (column 2b+kk holds W[256b + 2p + kk]).
        identm = const_pool.tile(
            [128, 128], BF16 if mm_dtype == F8 else mm_dtype
        )
        make_identity(nc, identm[:, :])
        e5476 = const_pool.tile([128, 1], F32)
        nc.vector.memset(e5476[:, :], float(TS))
        w1s = const_pool.tile([128, KC], F32)
        w2s = const_pool.tile([128, KC], F32)
        w1m = const_pool.tile([128, KC], mm_dtype)
        w2m = const_pool.tile([128, KC], mm_dtype)
        w3 = const_pool.tile([1, D], F32)
        bh = const_pool.tile([1, 1], F32)
        if mm_dtype == F8:
            for b in range(KC // 2):
                nc.sync.dma_start(
                    out=w1s[:, 2 * b:2 * b + 2],
                    in_=w_head[ds(256 * b, 256), :]
                    .rearrange("(p two) one -> p (two one)", two=2),
                )
                nc.sync.dma_start(
                    out=w2s[:, 2 * b:2 * b + 2],
                    in_=w_head[ds(D + 256 * b, 256), :]
                    .rearrange("(p two) one -> p (two one)", two=2),
                )
        else:
            for k in range(KC):
                nc.sync.dma_start(out=w1s[:, k:k + 1], in_=w_head[ds(128 * k, 128), :])
                nc.sync.dma_start(out=w2s[:, k:k + 1], in_=w_head[ds(D + 128 * k, 128), :])
        nc.scalar.mul(w1m[:, :], w1s[:, :], 16.0)
        nc.scalar.mul(w2m[:, :], w2s[:, :], 16.0)
        nc.sync.dma_start(out=w3[0:1, :], in_=w_head[ds(2 * D, D), :])
        nc.sync.dma_start(out=bh[:, :], in_=b_head[:, :])

        dummy_sq = const_pool.tile([128, D], F32)

        pools = (img_pool, raw_pool, f8_pool, scratch, simb_pool, psum_t,
                 psum_mm, dummy_sq, cls_pool)
        iota88u = const_pool.tile([128, 1], U32)
        iota88 = const_pool.tile([128, 1], F32)
        nc.gpsimd.iota(iota88u[:, :], pattern=[[0, 1]], base=0,
                       channel_multiplier=NWIN)
        nc.vector.tensor_copy(iota88[:, :], iota88u[:, :])
        consts = (identm, w1m, w2m, w3, bh, e5476, iota88)
        aps = (x_query, x_support, x_cls, p2d_list, c3d_list, simd_list, out0, out1)

        built = [None] * per_core
        for i in range(per_core):
            built[i] = _emit_image(nc, pools, consts, aps, i, mm_dtype)
        for i in range(per_core):
            _emit_sim(nc, pools, consts, aps, i, built[i], mm_dtype)

    nc.compile()
    return nc


_CACHED = {}


def _get_program(per_core=PER_CORE, mm_mode=MM_MODE):
    key = (per_core, mm_mode)
    if key not in _CACHED:
        _CACHED[key] = build_program(per_core, mm_mode)
    return _CACHED[key]


def run(inputs, trace=False, per_core=PER_CORE, mm_mode=MM_MODE):
    nc = _get_program(per_core, mm_mode)
    n_cores = N_FULL // per_core
    xq = np.ascontiguousarray(inputs["x_query"], dtype=np.float32)
    xs = np.ascontiguousarray(inputs["x_support"], dtype=np.float32)
    xc = np.ascontiguousarray(inputs["x_support_cls"], dtype=np.float32).reshape(
        N_FULL, S * D
    )
    wh = np.ascontiguousarray(inputs["W_head"], dtype=np.float32).reshape(3 * D, 1)
    bhv = np.ascontiguousarray(inputs["b_head"], dtype=np.float32).reshape(1, 1)
    in_maps = []
    for c in range(n_cores):
        sl = slice(c * per_core, (c + 1) * per_core)
        in_maps.append({
            "x_query": xq[sl], "x_support": xs[sl], "x_support_cls": xc[sl],
            "W_head": wh, "b_head": bhv,
        })
    res = run_bass_kernel_spmd(nc, in_maps, list(range(n_cores)), trace=trace)
    o0 = np.concatenate([res.results[c]["out0"] for c in range(n_cores)], axis=0)
    o1 = np.concatenate([res.results[c]["out1"] for c in range(n_cores)], axis=0)
    o0 = o0.reshape(N_FULL, 1, SIDE, SIDE).astype(np.float32)
    o1 = o1.reshape(N_FULL, 1, SIDE, SIDE).astype(np.float32)
    return (o0, o1), res


def kernel(**inputs):
    (o0, o1), _ = run(inputs, trace=False)
    return o0, o1
